# revision 37
# baseline (speedup 1.0000x reference)
"""Trainium2 Bass kernel for the shared-covariance Kalman filter problem.

Math: init_cov is identical for every group (the reference builds it as a
broadcast identity) and F/Q/H/R are shared, so the covariance (Riccati)
recursion — and with it every Kalman gain K_t and every measurement
covariance S_t — is group-independent.  The device runs ONE Riccati chain
(warm-started Newton-Schulz 16x16 inverse, exact transpose-pair form) with
the observation-driven mean recursion for all 128 groups riding on it, then
switches to a steady-state regime at TCUT where the gain has converged:
the remaining timesteps are processed 8-at-a-time with precomputed block
maps (one 96x96 + one 128x96 matmul per 8 steps instead of 8 small chains).

All 8 cores run the identical program (the problem is latency-bound, not
throughput-bound); outputs are taken from core 0.

Device-visible scalings (all folded so no extra ops are needed):
  Fs = sqrt(1/2) F        -> P' accumulation of 0.5(X+X^T) is 4 plain matmuls
  obs_pre = -sqrt(2) y    -> resid' = sqrt2*meas + obs_pre = -sqrt2(y - Hm)
  J = -sqrt(1/2)(F K)^T   -> J^T resid' = F K (y - Hm) exactly
"""
import numpy as np

import concourse.bacc as bacc
import concourse.bass as bass
import concourse.mybir as mybir
import concourse.tile as tile
from concourse.bass_utils import run_bass_kernel_spmd

G, T, S, M = 128, 512, 96, 16
TCUT = 320          # Riccati steps computed per-step; blocked steady beyond
TJ = 64             # switch to symmetric-Joseph regime (lagged gain) at this t
B = 8               # steady-phase block size (8*M = 128 = max contract dim)
CH = 64             # transient obs chunking along t
NCORES = 8
f32 = mybir.dt.float32
SQRT2 = float(np.sqrt(2.0))
Alu = mybir.AluOpType


def ns_iters(t: int) -> int:
    if t == 0:
        return 0        # X0 comes from the host, exact
    if t == 1:
        return 6
    if t == 2:
        return 4
    if t < 64:
        return 2
    return 1


def build_program(Tcut: int = TCUT, Ttot: int = T, ch: int = CH):
    ch = min(ch, Tcut)
    nblk = (Ttot - Tcut) // B
    assert Tcut + nblk * B == Ttot, "tail must be a multiple of B"
    nc = bacc.Bacc("TRN2", target_bir_lowering=False, debug=False,
                   num_devices=NCORES)

    P0_d = nc.declare_dram_parameter("P0", [S, S], f32, isOutput=False)
    mT0_d = nc.declare_dram_parameter("mT0", [S, G], f32, isOutput=False)
    FH_d = nc.declare_dram_parameter("FH", [S, S + M], f32, isOutput=False)
    Ft_d = nc.declare_dram_parameter("Ft", [S, S], f32, isOutput=False)
    Ff_d = nc.declare_dram_parameter("Ff", [S, S], f32, isOutput=False)
    Qs_d = nc.declare_dram_parameter("Qsym", [S, S], f32, isOutput=False)
    Hf_d = nc.declare_dram_parameter("Hf", [M, S], f32, isOutput=False)
    Rs_d = nc.declare_dram_parameter("Rsym", [M, M], f32, isOutput=False)
    I22_d = nc.declare_dram_parameter("I22", [M, 2 * M], f32, isOutput=False)
    X022_d = nc.declare_dram_parameter("X022", [M, 2 * M], f32, isOutput=False)
    obs_tr_d = nc.declare_dram_parameter("obs_tr", [Tcut, M, G], f32, isOutput=False)
    if nblk:
        obs_st_d = nc.declare_dram_parameter("obs_st", [B * M, nblk * G], f32,
                                             isOutput=False)
    resid_d = nc.declare_dram_parameter("out_resid", [M, Tcut, G], f32, isOutput=True)
    if nblk:
        mst_d = nc.declare_dram_parameter("out_mst", [nblk, B * M, G], f32,
                                          isOutput=True)
    covs_d = nc.declare_dram_parameter("out_covs", [M, Tcut, M], f32, isOutput=True)

    with tile.TileContext(nc) as tc:
        with (
            tc.tile_pool(name="consts", bufs=1) as cpool,
            tc.tile_pool(name="state", bufs=3) as spool,
            tc.tile_pool(name="work", bufs=2) as wpool,
            tc.tile_pool(name="store", bufs=1) as stpool,
            tc.tile_pool(name="io", bufs=2) as iopool,
            tc.tile_pool(name="ps_big", bufs=4, space="PSUM") as psb,
            tc.tile_pool(name="ps_small", bufs=4, space="PSUM") as pss,
        ):
            # ---- constants ----
            FH = cpool.tile([S, S + M], f32, tag="FH")       # [Fs^T | H^T]
            Ft = cpool.tile([S, S], f32, tag="Ft")
            Ff = cpool.tile([S, S], f32, tag="Ff")
            Qsym = cpool.tile([S, S], f32, tag="Qsym")
            Hf = cpool.tile([M, S], f32, tag="Hf")
            Rsym = cpool.tile([M, M], f32, tag="Rsym")
            I22 = cpool.tile([M, 2 * M], f32, tag="I22")     # [2I | 2I]
            nc.sync.dma_start(FH[:], FH_d[:])
            nc.sync.dma_start(Ft[:], Ft_d[:])
            nc.sync.dma_start(Ff[:], Ff_d[:])
            nc.sync.dma_start(Qsym[:], Qs_d[:])
            nc.sync.dma_start(Hf[:], Hf_d[:])
            nc.sync.dma_start(Rsym[:], Rs_d[:])
            nc.sync.dma_start(I22[:], I22_d[:])
            FsT = FH[:, :S]
            Ht = FH[:, S:]

            # ---- carried state ----
            P = spool.tile([S, S], f32, tag="P")
            nc.sync.dma_start(P[:], P0_d[:])
            X2 = spool.tile([M, 2 * M], f32, tag="X2")       # [Xm | Xm^T]
            nc.sync.dma_start(X2[:], X022_d[:])
            Xm = X2[:, :M]
            Xtm = X2[:, M:]
            # mr carries [m^T (96 rows); resid' (16 rows)]
            mr = spool.tile([S + M, G], f32, tag="mr")
            nc.sync.dma_start(mr[:S, :], mT0_d[:])
            # JF carries [F^T (96 rows); J_t (16 rows)] for the fused m-update
            JF = stpool.tile([S + M, S], f32, tag="JF")
            nc.vector.tensor_copy(JF[:S, :], Ft[:])

            # ---- covariance store ----
            Ss = stpool.tile([M, Tcut, M], f32, tag="Ss")

            tj = min(TJ, Tcut)
            AHb = Cb = None
            if Tcut > tj:
                # AH buffers hold [sqrt(.5)Acl^T | Fs^T | H^T] (last 112 cols const)
                AHb = [stpool.tile([S, 2 * S + M], f32, tag="AH0", name="AH0"),
                       stpool.tile([S, 2 * S + M], f32, tag="AH1", name="AH1")]
                Cb = [stpool.tile([S, S], f32, tag="C0", name="C0"),
                      stpool.tile([S, S], f32, tag="C1", name="C1")]
                nc.vector.tensor_copy(AHb[0][:, S:], FH[:])
                nc.vector.tensor_copy(AHb[1][:, S:], FH[:])

            def build_acl(Jsrc, idx):
                """Acl/C for the Joseph regime from a (lagged) gain J."""
                acl_ps = psb.tile([S, S], f32, tag="big", name="acl_ps")
                nc.tensor.matmul(acl_ps[:], Hf[:], Jsrc)       # H^T J
                nc.vector.tensor_add(AHb[idx][:, :S], acl_ps[:], FsT)  # sqrt(.5)Acl^T
                rj_ps = pss.tile([M, S], f32, tag="small", name="rj_ps")
                nc.tensor.matmul(rj_ps[:], Rsym[:], Jsrc)      # R J
                RJ = wpool.tile([M, S], f32, tag="RJ", name="RJ")
                nc.scalar.copy(RJ[:], rj_ps[:])
                c_ps = psb.tile([S, S], f32, tag="big", name="c_ps")
                nc.tensor.matmul(c_ps[:], Jsrc, RJ[:], start=True, stop=False)
                nc.tensor.matmul(c_ps[:], RJ[:], Jsrc, start=False, stop=True)
                nc.vector.tensor_add(Cb[idx][:], c_ps[:], Qsym[:])

            J = None
            Jprev = None
            HPF = None
            obs_ch = None

            def means_step(t):
                nonlocal mr, obs_ch
                if t % ch == 0:
                    csz = min(ch, Tcut - t)
                    obs_ch = iopool.tile([M, csz, G], f32, tag="obs", name="obs_ch")
                    nc.sync.dma_start(
                        obs_ch[:], obs_tr_d[t : t + csz].rearrange("t m g -> m t g")
                    )
                tl = t % ch
                meas_ps = pss.tile([M, G], f32, tag="small", name="meas_ps")
                nc.tensor.matmul(meas_ps[:], Ht, mr[:S, :])    # H m
                # resid' = sqrt2*meas + obs_pre, written into mr rows [S:S+M]
                nc.vector.scalar_tensor_tensor(
                    mr[S:, :], meas_ps[:], SQRT2, obs_ch[:, tl, :], Alu.mult, Alu.add
                )
                nc.sync.dma_start(resid_d[:, t, :], mr[S:, :])
                mu_ps = psb.tile([S, G], f32, tag="big", name="mu_ps")
                nc.tensor.matmul(mu_ps[:], JF[:], mr[:])       # F m + FK resid
                mr = spool.tile([S + M, G], f32, tag="mr", name="mr")
                nc.scalar.copy(mr[:S, :], mu_ps[:])

            def offchain_a(Wts, PHt):
                """PE-leading ops of the offchain block (no fresh deps)."""
                scov_ps = pss.tile([M, M], f32, tag="small", name="scov_ps")
                nc.tensor.matmul(scov_ps[:], PHt, Ht)          # H P H^T
                hpf_ps = pss.tile([M, S], f32, tag="small", name="hpf_ps")
                nc.tensor.matmul(hpf_ps[:], Ht, Wts)           # H P Fs^T
                return scov_ps, hpf_ps

            def offchain_b(t, scov_ps, hpf_ps):
                """Covariance output, NS refine, gain for step t."""
                nonlocal J, Jprev, HPF, Xm, Xtm
                Scv = Ss[:, t, :]
                nc.vector.tensor_add(Scv, scov_ps[:], Rsym[:])  # S_t
                HPF = wpool.tile([M, S], f32, tag="HPF", name="HPF")
                nc.scalar.copy(HPF[:], hpf_ps[:])
                for _ in range(ns_iters(t)):
                    v12 = pss.tile([M, 2 * M], f32, tag="small", name="v12")
                    nc.tensor.matmul(v12[:, :M], Scv, Xm)      # S Xm
                    nc.tensor.matmul(v12[:, M:], Xm, Scv)      # Xm^T S
                    Z2 = wpool.tile([M, 2 * M], f32, tag="Z2", name="Z2")
                    nc.vector.tensor_add(Z2[:], v12[:], I22[:])  # [Z | Z^T]
                    Zc = Z2[:, :M]
                    x12 = pss.tile([M, 2 * M], f32, tag="small", name="x12")
                    nc.tensor.matmul(x12[:, :M], Xtm, Zc)      # Xm Z
                    nc.tensor.matmul(x12[:, M:], Zc, Xtm)      # (Xm Z)^T
                    X2 = spool.tile([M, 2 * M], f32, tag="X2", name="X2")
                    nc.vector.tensor_copy(X2[:], x12[:])
                    Xm = X2[:, :M]
                    Xtm = X2[:, M:]
                j_ps = pss.tile([M, S], f32, tag="small", name="j_ps")
                nc.tensor.matmul(j_ps[:], Xtm, HPF[:])         # -Sinv H P Fs^T
                Jprev = J
                J = wpool.tile([M, S], f32, tag="J", name="J", bufs=3)
                nc.vector.tensor_copy(J[:], j_ps[:])
                nc.scalar.copy(JF[S:, :], j_ps[:])

            # ---------- transient regime (exact 4-mm symmetric update) ----------
            for t in range(tj):
                wp_ps = psb.tile([S, S + M], f32, tag="big", name="wp_ps")
                nc.tensor.matmul(wp_ps[:], P[:], FH[:])        # [P Fs^T | P H^T]
                WP = wpool.tile([S, S + M], f32, tag="WP", name="WP")
                nc.vector.tensor_copy(WP[:], wp_ps[:])
                Wts = WP[:, :S]
                PHt = WP[:, S:]
                if t > 0:
                    means_step(t - 1)      # pipelined one step behind
                offchain_b(t, *offchain_a(Wts, PHt))
                pacc = psb.tile([S, S], f32, tag="big", name="pacc")
                nc.tensor.matmul(pacc[:], Wts, FsT, start=True, stop=False)
                nc.tensor.matmul(pacc[:], FsT, Wts, start=False, stop=False)
                nc.tensor.matmul(pacc[:], HPF[:], J[:], start=False, stop=False)
                nc.tensor.matmul(pacc[:], J[:], HPF[:], start=False, stop=True)
                P = spool.tile([S, S], f32, tag="P", name="P")
                nc.vector.tensor_add(P[:], pacc[:], Qsym[:])   # P' = . + Q
                if t == tj - 1 and Tcut > tj:
                    build_acl(J[:], 0)

            # ---------- Joseph regime, software-pipelined emission ----------
            # step tj uses the transition build (buffer 0); pairs {tj+2k-1, tj+2k}
            # use buffer k%2, built from J_{tj+2k-3} (gain staleness 2/3 steps).
            def buf_for(t):
                return 0 if t == tj else (((t - tj + 1) // 2) % 2)

            pend = None
            for t in range(tj, Tcut):
                AHc = AHb[buf_for(t)]
                Cc = Cb[buf_for(t)]
                w1_ps = psb.tile([S, 2 * S + M], f32, tag="big", name="w1_ps")
                nc.tensor.matmul(w1_ps[:], P[:], AHc[:])   # [P Acl_s^T|P Fs^T|P H^T]
                W1 = wpool.tile([S, S], f32, tag="W1", name="W1")
                nc.vector.tensor_copy(W1[:], w1_ps[:, :S])
                WP = wpool.tile([S, S + M], f32, tag="WP", name="WP")
                nc.scalar.copy(WP[:], w1_ps[:, S:])
                pa = offchain_a(pend[1], pend[2]) if pend is not None else None
                pacc = psb.tile([S, S], f32, tag="big", name="pacc")
                nc.tensor.matmul(pacc[:], AHc[:, :S], W1[:], start=True, stop=False)
                nc.tensor.matmul(pacc[:], W1[:], AHc[:, :S], start=False, stop=True)
                Pn = spool.tile([S, S], f32, tag="P", name="P")
                nc.vector.tensor_add(Pn[:], pacc[:], Cc[:])  # P' = sym + C
                if pend is not None:
                    offchain_b(pend[0], *pa)   # previous step's output/gain work
                    means_step(pend[0])
                elif t == tj:
                    means_step(tj - 1)         # last transient means
                if (t - tj) % 2 == 0 and t + 1 < Tcut:
                    # J here is J_{t-1} (just emitted by offchain, or the
                    # transition J at t == tj); serves steps {t+1, t+2}
                    k = (t - tj) // 2 + 1
                    build_acl(J[:], k % 2)
                pend = (t, WP[:, :S], WP[:, S:])
                P = Pn
            if pend is not None:
                offchain_b(pend[0], *offchain_a(pend[1], pend[2]))
                means_step(pend[0])
            elif tj > 0:
                means_step(tj - 1)

            nc.sync.dma_start(covs_d[:], Ss[:])

            if nblk:
                # ---------- steady-phase block maps ----------
                sq2 = SQRT2
                # Jt = J^T  (exact transpose via the pair trick)
                jt_ps = psb.tile([S, M], f32, tag="big", name="jt_ps")
                nc.tensor.matmul(jt_ps[:], HPF[:], Xtm)
                Jt = wpool.tile([S, M], f32, tag="Jt", name="Jt")
                nc.vector.tensor_copy(Jt[:], jt_ps[:])
                # A = F + sqrt2 J^T H ; A^T = F^T + sqrt2 H^T J
                an_ps = psb.tile([S, S], f32, tag="big", name="an_ps")
                nc.tensor.matmul(an_ps[:], J[:], Hf[:])
                An = wpool.tile([S, S], f32, tag="An", name="An")
                nc.vector.scalar_tensor_tensor(An[:], an_ps[:], sq2, Ff[:],
                                               Alu.mult, Alu.add)
                at_ps = psb.tile([S, S], f32, tag="big", name="at_ps")
                nc.tensor.matmul(at_ps[:], Hf[:], J[:])
                At = wpool.tile([S, S], f32, tag="At", name="At")
                nc.vector.scalar_tensor_tensor(At[:], at_ps[:], sq2, Ft[:],
                                               Alu.mult, Alu.add)
                # power pairs A^k, (A^k)^T for k=2..8
                apow = {1: (An, At)}
                for k in range(2, B + 1):
                    prev_n, _ = apow[k - 1]
                    pn_ps = psb.tile([S, S], f32, tag="big", name="pn_ps")
                    nc.tensor.matmul(pn_ps[:], At[:], prev_n[:])   # A A^{k-1}
                    pt_ps = psb.tile([S, S], f32, tag="big", name="pt_ps")
                    nc.tensor.matmul(pt_ps[:], prev_n[:], At[:])   # (A^k)^T
                    an_k = wpool.tile([S, S], f32, tag=f"A{k}n", name=f"A{k}n")
                    at_k = wpool.tile([S, S], f32, tag=f"A{k}t", name=f"A{k}t")
                    nc.vector.tensor_copy(an_k[:], pn_ps[:])
                    nc.scalar.copy(at_k[:], pt_ps[:])
                    apow[k] = (an_k, at_k)
                A8t = apow[B][1]
                # G_k = J A^kT [M,S], AkJt_k = A^k J^T [S,M]
                gks = {0: J}
                akjts = {0: Jt}
                for k in range(1, B):
                    _, at_k = apow[k]
                    g_ps = pss.tile([M, S], f32, tag="small", name="g_ps")
                    nc.tensor.matmul(g_ps[:], Jt[:], at_k[:])
                    gk = wpool.tile([M, S], f32, tag=f"G{k}", name=f"G{k}")
                    nc.vector.tensor_copy(gk[:], g_ps[:])
                    gks[k] = gk
                    aj_ps = psb.tile([S, M], f32, tag="big", name="aj_ps")
                    nc.tensor.matmul(aj_ps[:], at_k[:], Jt[:])
                    akjt = wpool.tile([S, M], f32, tag=f"AJ{k}", name=f"AJ{k}")
                    nc.scalar.copy(akjt[:], aj_ps[:])
                    akjts[k] = akjt
                # Wstack [(j,m), S]: row-block j = G_{B-1-j}  (partition moves -> DMA)
                Wst = stpool.tile([B * M, S], f32, tag="Wst")
                for j in range(B):
                    nc.sync.dma_start(Wst[j * M : (j + 1) * M, :], gks[B - 1 - j][:])
                # Et [S, (j,m)]: col-block j = A^jT H^T
                Et = stpool.tile([S, B * M], f32, tag="Et")
                nc.vector.tensor_copy(Et[:, :M], Ht)
                for j in range(1, B):
                    aj_n, _ = apow[j]
                    e_ps = psb.tile([S, M], f32, tag="big", name="e_ps")
                    nc.tensor.matmul(e_ps[:], aj_n[:], Ht)
                    nc.vector.tensor_copy(Et[:, j * M : (j + 1) * M], e_ps[:])
                # Ct [(i,m'), (j,m)]: block (i,j) = J A^(j-1-i)T H^T
                Ct = stpool.tile([B * M, B * M], f32, tag="Ct")
                nc.gpsimd.memset(Ct[:], 0.0)
                cblk = {}
                for k in range(0, B - 1):
                    c_ps = pss.tile([M, M], f32, tag="small", name="c_ps")
                    nc.tensor.matmul(c_ps[:], akjts[k][:], Ht)
                    cb = wpool.tile([M, M], f32, tag=f"C{k}", name=f"C{k}")
                    nc.vector.tensor_copy(cb[:], c_ps[:])
                    cblk[k] = cb
                for i in range(B):
                    for j in range(i + 1, B):
                        nc.sync.dma_start(
                            Ct[i * M : (i + 1) * M, j * M : (j + 1) * M],
                            cblk[j - 1 - i][:],
                        )
                # steady observations, one DMA
                ybig = stpool.tile([B * M, nblk, G], f32, tag="ybig")
                nc.sync.dma_start(
                    ybig[:], obs_st_d[:].rearrange("p (b g) -> p b g", g=G)
                )

                # ---------- steady block loop ----------
                mcur = mr[:S, :]
                for b in range(nblk):
                    yb = ybig[:, b, :]
                    me_ps = psb.tile([B * M, G], f32, tag="big", name="me_ps")
                    nc.tensor.matmul(me_ps[:], Et[:], mcur, start=True, stop=False)
                    nc.tensor.matmul(me_ps[:], Ct[:], yb, start=False, stop=True)
                    msb = wpool.tile([B * M, G], f32, tag="msb", name="msb", bufs=3)
                    nc.scalar.copy(msb[:], me_ps[:])
                    nc.sync.dma_start(mst_d[b], msb[:])
                    ma_ps = psb.tile([S, G], f32, tag="big", name="ma_ps")
                    nc.tensor.matmul(ma_ps[:], A8t[:], mcur, start=True, stop=False)
                    nc.tensor.matmul(ma_ps[:], Wst[:], yb, start=False, stop=True)
                    mS = spool.tile([S, G], f32, tag="mS", name="mS")
                    nc.vector.tensor_copy(mS[:], ma_ps[:])
                    mcur = mS[:]

    nc.compile()
    return nc


_program_cache = {}


def _get_program(Tcut, Ttot):
    key = (Tcut, Ttot)
    if key not in _program_cache:
        _program_cache[key] = build_program(Tcut, Ttot)
    return _program_cache[key]


def _host_precompute(obs, F, Q, H, R, init_mean, init_cov, Tcut=TCUT, Ttot=T):
    f = np.float32
    nblk = (Ttot - Tcut) // B
    Fsc = np.sqrt(0.5).astype(f) * F.astype(f)
    P0 = init_cov[0].astype(f)
    Qsym = (0.5 * (Q + Q.T)).astype(f)
    Rsym = (0.5 * (R + R.T)).astype(f)
    S0 = (H.astype(np.float64) @ P0.astype(np.float64) @ H.T.astype(np.float64)
          + Rsym.astype(np.float64))
    X0 = np.linalg.inv(S0)
    X0m = (-X0).astype(f)
    I16 = np.eye(M, dtype=f)
    obs_pre = ((-np.sqrt(2.0)).astype(f) * obs.astype(f)).transpose(1, 2, 0)  # [T,M,G]
    ins = {
        "P0": P0,
        "mT0": np.ascontiguousarray(init_mean.astype(f).T),
        "FH": np.ascontiguousarray(np.concatenate([Fsc.T, H.astype(f).T], axis=1)),
        "Ft": np.ascontiguousarray(F.astype(f).T),
        "Ff": np.ascontiguousarray(F.astype(f)),
        "Qsym": Qsym,
        "Hf": np.ascontiguousarray(H.astype(f)),
        "Rsym": Rsym,
        "I22": np.ascontiguousarray(np.concatenate([2 * I16, 2 * I16], axis=1)),
        "X022": np.ascontiguousarray(np.concatenate([X0m, X0m.T], axis=1)),
        "obs_tr": np.ascontiguousarray(obs_pre[:Tcut]),
    }
    if nblk:
        ins["obs_st"] = np.ascontiguousarray(
            obs_pre[Tcut:Ttot].reshape(nblk, B, M, G).transpose(1, 2, 0, 3)
            .reshape(B * M, nblk * G)
        )
    return ins


def _run_device(inputs, trace=False, tmpdir=None, Tcut=TCUT, Ttot=T):
    obs, F, Q, H, R = (inputs["obs"], inputs["F"], inputs["Q"], inputs["H"],
                       inputs["R"])
    init_mean, init_cov = inputs["init_mean"], inputs["init_cov"]
    nblk = (Ttot - Tcut) // B
    nc = _get_program(Tcut, Ttot)
    ins = _host_precompute(obs, F, Q, H, R, init_mean, init_cov, Tcut, Ttot)
    in_maps = [dict(ins) for _ in range(NCORES)]
    res = run_bass_kernel_spmd(
        nc, in_maps, list(range(NCORES)), trace=trace, tmpdir=tmpdir
    )
    out = res.results[0]
    f = np.float32
    means = np.empty((G, Ttot, M), f)
    # transient: meas = resid'/sqrt2 + y
    resid = out["out_resid"]                        # [M, Tcut, G]
    means[:, :Tcut, :] = (resid.transpose(2, 1, 0) / np.sqrt(2.0).astype(f)
                          + obs[:, :Tcut, :].astype(f))
    if nblk:
        mst = out["out_mst"]                        # [nblk, B*M, G]
        means[:, Tcut:, :] = (mst.reshape(nblk, B, M, G).transpose(3, 0, 1, 2)
                              .reshape(G, Ttot - Tcut, M))
    covs_mtm = out["out_covs"]                      # [M, Tcut, M]
    ss = covs_mtm.transpose(1, 0, 2)                # [Tcut, M, M]
    covs_t = np.empty((Ttot, M, M), f)
    covs_t[:Tcut] = ss
    covs_t[Tcut:] = ss[Tcut - 1]
    covs = np.broadcast_to(covs_t[None], (G, Ttot, M, M)).copy()
    return means, covs, res


def _reference_fallback(obs, F, Q, H, R, init_mean, init_cov):
    """Exact per-group filter in float64 — used only if assumptions fail."""
    d = np.float64
    obs, F, Q, H, R = obs.astype(d), F.astype(d), Q.astype(d), H.astype(d), R.astype(d)
    m = init_mean.astype(d)
    P = 0.5 * (init_cov.astype(d) + init_cov.astype(d).transpose(0, 2, 1))
    g, tt, mm_ = obs.shape
    s = F.shape[0]
    means = np.zeros((g, tt, mm_), np.float32)
    covs = np.zeros((g, tt, mm_, mm_), np.float32)
    I = np.eye(s)
    for t in range(tt):
        meas_mean = m @ H.T
        PHt = P @ H.T
        meas_cov = np.einsum("ms,gsn->gmn", H, PHt) + R
        means[:, t] = meas_mean
        covs[:, t] = meas_cov
        Kt = np.linalg.solve(meas_cov, PHt.transpose(0, 2, 1))
        K = Kt.transpose(0, 2, 1)
        resid = obs[:, t] - meas_mean
        m_upd = m + np.einsum("gsm,gm->gs", K, resid)
        P_upd = (I[None] - K @ H) @ P
        m = m_upd @ F.T
        X = np.einsum("ij,gjk,lk->gil", F, P_upd, F) + Q
        P = 0.5 * (X + X.transpose(0, 2, 1))
    return means, covs


def kernel(obs, F, Q, H, R, init_mean, init_cov):
    obs = np.asarray(obs)
    F = np.asarray(F); Q = np.asarray(Q); H = np.asarray(H); R = np.asarray(R)
    init_mean = np.asarray(init_mean); init_cov = np.asarray(init_cov)

    shapes_ok = (
        obs.shape == (G, T, M) and F.shape == (S, S) and H.shape == (M, S)
        and init_cov.shape == (G, S, S)
    )
    uniform_cov = shapes_ok and bool(np.all(init_cov == init_cov[0]))
    if not uniform_cov:
        means, covs = _reference_fallback(obs, F, Q, H, R, init_mean, init_cov)
        return means, covs

    try:
        means, covs, _ = _run_device(
            dict(obs=obs, F=F, Q=Q, H=H, R=R, init_mean=init_mean, init_cov=init_cov)
        )
        if not (np.isfinite(means).all() and np.isfinite(covs).all()):
            raise FloatingPointError("non-finite device output")
    except Exception:
        means, covs = _reference_fallback(obs, F, Q, H, R, init_mean, init_cov)
    return means, covs


# revision 38
# speedup vs baseline: 1.0485x; 1.0485x over previous
"""Trainium2 Bass kernel for the shared-covariance Kalman filter problem.

Math: init_cov is identical for every group (the reference builds it as a
broadcast identity) and F/Q/H/R are shared, so the covariance (Riccati)
recursion — and with it every Kalman gain K_t and every measurement
covariance S_t — is group-independent.  The device runs ONE Riccati chain
(warm-started Newton-Schulz 16x16 inverse, exact transpose-pair form) with
the observation-driven mean recursion for all 128 groups riding on it, then
switches to a steady-state regime at TCUT where the gain has converged:
the remaining timesteps are processed 8-at-a-time with precomputed block
maps (one 96x96 + one 128x96 matmul per 8 steps instead of 8 small chains).

All 8 cores run the identical program (the problem is latency-bound, not
throughput-bound); outputs are taken from core 0.

Device-visible scalings (all folded so no extra ops are needed):
  Fs = sqrt(1/2) F        -> P' accumulation of 0.5(X+X^T) is 4 plain matmuls
  obs_pre = -sqrt(2) y    -> resid' = sqrt2*meas + obs_pre = -sqrt2(y - Hm)
  J = -sqrt(1/2)(F K)^T   -> J^T resid' = F K (y - Hm) exactly
"""
import numpy as np

import concourse.bacc as bacc
import concourse.bass as bass
import concourse.mybir as mybir
import concourse.tile as tile
from concourse.bass_utils import run_bass_kernel_spmd

G, T, S, M = 128, 512, 96, 16
TCUT = 320          # Riccati steps computed per-step; blocked steady beyond
TJ = 64             # switch to symmetric-Joseph regime (lagged gain) at this t
B = 8               # steady-phase block size (8*M = 128 = max contract dim)
CH = 64             # transient obs chunking along t
NCORES = 8
f32 = mybir.dt.float32
SQRT2 = float(np.sqrt(2.0))
Alu = mybir.AluOpType


def ns_iters(t: int) -> int:
    if t == 0:
        return 0        # X0 comes from the host, exact
    if t == 1:
        return 6
    if t == 2:
        return 4
    if t < 16:
        return 2
    return 1


def build_program(Tcut: int = TCUT, Ttot: int = T, ch: int = CH):
    ch = min(ch, Tcut)
    nblk = (Ttot - Tcut) // B
    assert Tcut + nblk * B == Ttot, "tail must be a multiple of B"
    nc = bacc.Bacc("TRN2", target_bir_lowering=False, debug=False,
                   num_devices=NCORES)

    P0_d = nc.declare_dram_parameter("P0", [S, S], f32, isOutput=False)
    mT0_d = nc.declare_dram_parameter("mT0", [S, G], f32, isOutput=False)
    FH_d = nc.declare_dram_parameter("FH", [S, S + M], f32, isOutput=False)
    Ft_d = nc.declare_dram_parameter("Ft", [S, S], f32, isOutput=False)
    Ff_d = nc.declare_dram_parameter("Ff", [S, S], f32, isOutput=False)
    Qs_d = nc.declare_dram_parameter("Qsym", [S, S], f32, isOutput=False)
    Hf_d = nc.declare_dram_parameter("Hf", [M, S], f32, isOutput=False)
    Rs_d = nc.declare_dram_parameter("Rsym", [M, M], f32, isOutput=False)
    I22_d = nc.declare_dram_parameter("I22", [M, 2 * M], f32, isOutput=False)
    X022_d = nc.declare_dram_parameter("X022", [M, 2 * M], f32, isOutput=False)
    obs_tr_d = nc.declare_dram_parameter("obs_tr", [Tcut, M, G], f32, isOutput=False)
    if nblk:
        obs_st_d = nc.declare_dram_parameter("obs_st", [B * M, nblk * G], f32,
                                             isOutput=False)
    resid_d = nc.declare_dram_parameter("out_resid", [M, Tcut, G], f32, isOutput=True)
    if nblk:
        mst_d = nc.declare_dram_parameter("out_mst", [nblk, B * M, G], f32,
                                          isOutput=True)
    covs_d = nc.declare_dram_parameter("out_covs", [M, Tcut, M], f32, isOutput=True)

    with tile.TileContext(nc) as tc:
        with (
            tc.tile_pool(name="consts", bufs=1) as cpool,
            tc.tile_pool(name="state", bufs=3) as spool,
            tc.tile_pool(name="work", bufs=2) as wpool,
            tc.tile_pool(name="store", bufs=1) as stpool,
            tc.tile_pool(name="io", bufs=2) as iopool,
            tc.tile_pool(name="ps_big", bufs=4, space="PSUM") as psb,
            tc.tile_pool(name="ps_small", bufs=4, space="PSUM") as pss,
        ):
            # ---- constants ----
            FH = cpool.tile([S, S + M], f32, tag="FH")       # [Fs^T | H^T]
            Ft = cpool.tile([S, S], f32, tag="Ft")
            Ff = cpool.tile([S, S], f32, tag="Ff")
            Qsym = cpool.tile([S, S], f32, tag="Qsym")
            Hf = cpool.tile([M, S], f32, tag="Hf")
            Rsym = cpool.tile([M, M], f32, tag="Rsym")
            I22 = cpool.tile([M, 2 * M], f32, tag="I22")     # [2I | 2I]
            nc.sync.dma_start(FH[:], FH_d[:])
            nc.sync.dma_start(Ft[:], Ft_d[:])
            nc.sync.dma_start(Ff[:], Ff_d[:])
            nc.sync.dma_start(Qsym[:], Qs_d[:])
            nc.sync.dma_start(Hf[:], Hf_d[:])
            nc.sync.dma_start(Rsym[:], Rs_d[:])
            nc.sync.dma_start(I22[:], I22_d[:])
            FsT = FH[:, :S]
            Ht = FH[:, S:]

            # ---- carried state ----
            P = spool.tile([S, S], f32, tag="P")
            nc.sync.dma_start(P[:], P0_d[:])
            X2 = spool.tile([M, 2 * M], f32, tag="X2")       # [Xm | Xm^T]
            nc.sync.dma_start(X2[:], X022_d[:])
            Xm = X2[:, :M]
            Xtm = X2[:, M:]
            # mr carries [m^T (96 rows); resid' (16 rows)]
            mr = spool.tile([S + M, G], f32, tag="mr")
            nc.sync.dma_start(mr[:S, :], mT0_d[:])
            # JF carries [F^T (96 rows); J_t (16 rows)] for the fused m-update
            JF = stpool.tile([S + M, S], f32, tag="JF")
            nc.vector.tensor_copy(JF[:S, :], Ft[:])

            # ---- covariance store ----
            Ss = stpool.tile([M, Tcut, M], f32, tag="Ss")

            tj = min(TJ, Tcut)
            AHb = Cb = None
            if Tcut > tj:
                # AH buffers hold [sqrt(.5)Acl^T | Fs^T | H^T] (last 112 cols const)
                AHb = [stpool.tile([S, 2 * S + M], f32, tag="AH0", name="AH0"),
                       stpool.tile([S, 2 * S + M], f32, tag="AH1", name="AH1")]
                Cb = [stpool.tile([S, S], f32, tag="C0", name="C0"),
                      stpool.tile([S, S], f32, tag="C1", name="C1")]
                nc.vector.tensor_copy(AHb[0][:, S:], FH[:])
                nc.vector.tensor_copy(AHb[1][:, S:], FH[:])

            def build_acl(Jsrc, idx):
                """Acl/C for the Joseph regime from a (lagged) gain J."""
                acl_ps = psb.tile([S, S], f32, tag="big", name="acl_ps")
                nc.tensor.matmul(acl_ps[:], Hf[:], Jsrc)       # H^T J
                nc.vector.tensor_add(AHb[idx][:, :S], acl_ps[:], FsT)  # sqrt(.5)Acl^T
                rj_ps = pss.tile([M, S], f32, tag="small", name="rj_ps")
                nc.tensor.matmul(rj_ps[:], Rsym[:], Jsrc)      # R J
                RJ = wpool.tile([M, S], f32, tag="RJ", name="RJ")
                nc.scalar.copy(RJ[:], rj_ps[:])
                c_ps = psb.tile([S, S], f32, tag="big", name="c_ps")
                nc.tensor.matmul(c_ps[:], Jsrc, RJ[:], start=True, stop=False)
                nc.tensor.matmul(c_ps[:], RJ[:], Jsrc, start=False, stop=True)
                nc.vector.tensor_add(Cb[idx][:], c_ps[:], Qsym[:])

            J = None
            Jprev = None
            HPF = None
            obs_ch = None

            def means_step(t):
                nonlocal mr, obs_ch
                if t % ch == 0:
                    csz = min(ch, Tcut - t)
                    obs_ch = iopool.tile([M, csz, G], f32, tag="obs", name="obs_ch")
                    nc.sync.dma_start(
                        obs_ch[:], obs_tr_d[t : t + csz].rearrange("t m g -> m t g")
                    )
                tl = t % ch
                meas_ps = pss.tile([M, G], f32, tag="small", name="meas_ps")
                nc.tensor.matmul(meas_ps[:], Ht, mr[:S, :])    # H m
                # resid' = sqrt2*meas + obs_pre, written into mr rows [S:S+M]
                nc.vector.scalar_tensor_tensor(
                    mr[S:, :], meas_ps[:], SQRT2, obs_ch[:, tl, :], Alu.mult, Alu.add
                )
                nc.sync.dma_start(resid_d[:, t, :], mr[S:, :])
                mu_ps = psb.tile([S, G], f32, tag="big", name="mu_ps")
                nc.tensor.matmul(mu_ps[:], JF[:], mr[:])       # F m + FK resid
                mr = spool.tile([S + M, G], f32, tag="mr", name="mr")
                nc.scalar.copy(mr[:S, :], mu_ps[:])

            def offchain_a(Wts, PHt):
                """PE-leading ops of the offchain block (no fresh deps)."""
                scov_ps = pss.tile([M, M], f32, tag="small", name="scov_ps")
                nc.tensor.matmul(scov_ps[:], PHt, Ht)          # H P H^T
                hpf_ps = pss.tile([M, S], f32, tag="small", name="hpf_ps")
                nc.tensor.matmul(hpf_ps[:], Ht, Wts)           # H P Fs^T
                return scov_ps, hpf_ps

            def offchain_b(t, scov_ps, hpf_ps):
                """Covariance output, NS refine, gain for step t."""
                nonlocal J, Jprev, HPF, Xm, Xtm
                Scv = Ss[:, t, :]
                nc.vector.tensor_add(Scv, scov_ps[:], Rsym[:])  # S_t
                HPF = wpool.tile([M, S], f32, tag="HPF", name="HPF")
                nc.scalar.copy(HPF[:], hpf_ps[:])
                for _ in range(ns_iters(t)):
                    v12 = pss.tile([M, 2 * M], f32, tag="small", name="v12")
                    nc.tensor.matmul(v12[:, :M], Scv, Xm)      # S Xm
                    nc.tensor.matmul(v12[:, M:], Xm, Scv)      # Xm^T S
                    Z2 = wpool.tile([M, 2 * M], f32, tag="Z2", name="Z2")
                    nc.vector.tensor_add(Z2[:], v12[:], I22[:])  # [Z | Z^T]
                    Zc = Z2[:, :M]
                    x12 = pss.tile([M, 2 * M], f32, tag="small", name="x12")
                    nc.tensor.matmul(x12[:, :M], Xtm, Zc)      # Xm Z
                    nc.tensor.matmul(x12[:, M:], Zc, Xtm)      # (Xm Z)^T
                    X2 = spool.tile([M, 2 * M], f32, tag="X2", name="X2")
                    nc.vector.tensor_copy(X2[:], x12[:])
                    Xm = X2[:, :M]
                    Xtm = X2[:, M:]
                j_ps = pss.tile([M, S], f32, tag="small", name="j_ps")
                nc.tensor.matmul(j_ps[:], Xtm, HPF[:])         # -Sinv H P Fs^T
                Jprev = J
                J = wpool.tile([M, S], f32, tag="J", name="J", bufs=3)
                nc.vector.tensor_copy(J[:], j_ps[:])
                nc.scalar.copy(JF[S:, :], j_ps[:])

            # ---------- transient regime (exact 4-mm symmetric update) ----------
            for t in range(tj):
                wp_ps = psb.tile([S, S + M], f32, tag="big", name="wp_ps")
                nc.tensor.matmul(wp_ps[:], P[:], FH[:])        # [P Fs^T | P H^T]
                WP = wpool.tile([S, S + M], f32, tag="WP", name="WP")
                nc.vector.tensor_copy(WP[:], wp_ps[:])
                Wts = WP[:, :S]
                PHt = WP[:, S:]
                if t > 0:
                    means_step(t - 1)      # pipelined one step behind
                offchain_b(t, *offchain_a(Wts, PHt))
                pacc = psb.tile([S, S], f32, tag="big", name="pacc")
                nc.tensor.matmul(pacc[:], Wts, FsT, start=True, stop=False)
                nc.tensor.matmul(pacc[:], FsT, Wts, start=False, stop=False)
                nc.tensor.matmul(pacc[:], HPF[:], J[:], start=False, stop=False)
                nc.tensor.matmul(pacc[:], J[:], HPF[:], start=False, stop=True)
                P = spool.tile([S, S], f32, tag="P", name="P")
                nc.vector.tensor_add(P[:], pacc[:], Qsym[:])   # P' = . + Q
                if t == tj - 1 and Tcut > tj:
                    build_acl(J[:], 0)

            # ---------- Joseph regime, software-pipelined emission ----------
            # step tj uses the transition build (buffer 0); pairs {tj+2k-1, tj+2k}
            # use buffer k%2, built from J_{tj+2k-3} (gain staleness 2/3 steps).
            def buf_for(t):
                return 0 if t == tj else (((t - tj + 1) // 2) % 2)

            pend = None
            for t in range(tj, Tcut):
                AHc = AHb[buf_for(t)]
                Cc = Cb[buf_for(t)]
                w1_ps = psb.tile([S, 2 * S + M], f32, tag="big", name="w1_ps")
                nc.tensor.matmul(w1_ps[:], P[:], AHc[:])   # [P Acl_s^T|P Fs^T|P H^T]
                W1 = wpool.tile([S, S], f32, tag="W1", name="W1")
                nc.vector.tensor_copy(W1[:], w1_ps[:, :S])
                WP = wpool.tile([S, S + M], f32, tag="WP", name="WP")
                nc.scalar.copy(WP[:], w1_ps[:, S:])
                pa = offchain_a(pend[1], pend[2]) if pend is not None else None
                pacc = psb.tile([S, S], f32, tag="big", name="pacc")
                nc.tensor.matmul(pacc[:], AHc[:, :S], W1[:], start=True, stop=False)
                nc.tensor.matmul(pacc[:], W1[:], AHc[:, :S], start=False, stop=True)
                Pn = spool.tile([S, S], f32, tag="P", name="P")
                nc.vector.tensor_add(Pn[:], pacc[:], Cc[:])  # P' = sym + C
                if pend is not None:
                    offchain_b(pend[0], *pa)   # previous step's output/gain work
                    means_step(pend[0])
                elif t == tj:
                    means_step(tj - 1)         # last transient means
                if (t - tj) % 2 == 0 and t + 1 < Tcut:
                    # J here is J_{t-1} (just emitted by offchain, or the
                    # transition J at t == tj); serves steps {t+1, t+2}
                    k = (t - tj) // 2 + 1
                    build_acl(J[:], k % 2)
                pend = (t, WP[:, :S], WP[:, S:])
                P = Pn
            if pend is not None:
                offchain_b(pend[0], *offchain_a(pend[1], pend[2]))
                means_step(pend[0])
            elif tj > 0:
                means_step(tj - 1)

            nc.sync.dma_start(covs_d[:], Ss[:])

            if nblk:
                # ---------- steady-phase block maps ----------
                sq2 = SQRT2
                # Jt = J^T  (exact transpose via the pair trick)
                jt_ps = psb.tile([S, M], f32, tag="big", name="jt_ps")
                nc.tensor.matmul(jt_ps[:], HPF[:], Xtm)
                Jt = wpool.tile([S, M], f32, tag="Jt", name="Jt")
                nc.vector.tensor_copy(Jt[:], jt_ps[:])
                # A = F + sqrt2 J^T H ; A^T = F^T + sqrt2 H^T J
                an_ps = psb.tile([S, S], f32, tag="big", name="an_ps")
                nc.tensor.matmul(an_ps[:], J[:], Hf[:])
                An = wpool.tile([S, S], f32, tag="An", name="An")
                nc.vector.scalar_tensor_tensor(An[:], an_ps[:], sq2, Ff[:],
                                               Alu.mult, Alu.add)
                at_ps = psb.tile([S, S], f32, tag="big", name="at_ps")
                nc.tensor.matmul(at_ps[:], Hf[:], J[:])
                At = wpool.tile([S, S], f32, tag="At", name="At")
                nc.vector.scalar_tensor_tensor(At[:], at_ps[:], sq2, Ft[:],
                                               Alu.mult, Alu.add)
                # power pairs A^k, (A^k)^T for k=2..8
                apow = {1: (An, At)}
                for k in range(2, B + 1):
                    prev_n, _ = apow[k - 1]
                    pn_ps = psb.tile([S, S], f32, tag="big", name="pn_ps")
                    nc.tensor.matmul(pn_ps[:], At[:], prev_n[:])   # A A^{k-1}
                    pt_ps = psb.tile([S, S], f32, tag="big", name="pt_ps")
                    nc.tensor.matmul(pt_ps[:], prev_n[:], At[:])   # (A^k)^T
                    an_k = wpool.tile([S, S], f32, tag=f"A{k}n", name=f"A{k}n")
                    at_k = wpool.tile([S, S], f32, tag=f"A{k}t", name=f"A{k}t")
                    nc.vector.tensor_copy(an_k[:], pn_ps[:])
                    nc.scalar.copy(at_k[:], pt_ps[:])
                    apow[k] = (an_k, at_k)
                A8t = apow[B][1]
                # G_k = J A^kT [M,S], AkJt_k = A^k J^T [S,M]
                gks = {0: J}
                akjts = {0: Jt}
                for k in range(1, B):
                    _, at_k = apow[k]
                    g_ps = pss.tile([M, S], f32, tag="small", name="g_ps")
                    nc.tensor.matmul(g_ps[:], Jt[:], at_k[:])
                    gk = wpool.tile([M, S], f32, tag=f"G{k}", name=f"G{k}")
                    nc.vector.tensor_copy(gk[:], g_ps[:])
                    gks[k] = gk
                    aj_ps = psb.tile([S, M], f32, tag="big", name="aj_ps")
                    nc.tensor.matmul(aj_ps[:], at_k[:], Jt[:])
                    akjt = wpool.tile([S, M], f32, tag=f"AJ{k}", name=f"AJ{k}")
                    nc.scalar.copy(akjt[:], aj_ps[:])
                    akjts[k] = akjt
                # Wstack [(j,m), S]: row-block j = G_{B-1-j}  (partition moves -> DMA)
                Wst = stpool.tile([B * M, S], f32, tag="Wst")
                for j in range(B):
                    nc.sync.dma_start(Wst[j * M : (j + 1) * M, :], gks[B - 1 - j][:])
                # Et [S, (j,m)]: col-block j = A^jT H^T
                Et = stpool.tile([S, B * M], f32, tag="Et")
                nc.vector.tensor_copy(Et[:, :M], Ht)
                for j in range(1, B):
                    aj_n, _ = apow[j]
                    e_ps = psb.tile([S, M], f32, tag="big", name="e_ps")
                    nc.tensor.matmul(e_ps[:], aj_n[:], Ht)
                    nc.vector.tensor_copy(Et[:, j * M : (j + 1) * M], e_ps[:])
                # Ct [(i,m'), (j,m)]: block (i,j) = J A^(j-1-i)T H^T
                Ct = stpool.tile([B * M, B * M], f32, tag="Ct")
                nc.gpsimd.memset(Ct[:], 0.0)
                cblk = {}
                for k in range(0, B - 1):
                    c_ps = pss.tile([M, M], f32, tag="small", name="c_ps")
                    nc.tensor.matmul(c_ps[:], akjts[k][:], Ht)
                    cb = wpool.tile([M, M], f32, tag=f"C{k}", name=f"C{k}")
                    nc.vector.tensor_copy(cb[:], c_ps[:])
                    cblk[k] = cb
                for i in range(B):
                    for j in range(i + 1, B):
                        nc.sync.dma_start(
                            Ct[i * M : (i + 1) * M, j * M : (j + 1) * M],
                            cblk[j - 1 - i][:],
                        )
                # steady observations, one DMA
                ybig = stpool.tile([B * M, nblk, G], f32, tag="ybig")
                nc.sync.dma_start(
                    ybig[:], obs_st_d[:].rearrange("p (b g) -> p b g", g=G)
                )

                # ---------- steady block loop ----------
                mcur = mr[:S, :]
                for b in range(nblk):
                    yb = ybig[:, b, :]
                    me_ps = psb.tile([B * M, G], f32, tag="big", name="me_ps")
                    nc.tensor.matmul(me_ps[:], Et[:], mcur, start=True, stop=False)
                    nc.tensor.matmul(me_ps[:], Ct[:], yb, start=False, stop=True)
                    msb = wpool.tile([B * M, G], f32, tag="msb", name="msb", bufs=3)
                    nc.scalar.copy(msb[:], me_ps[:])
                    nc.sync.dma_start(mst_d[b], msb[:])
                    ma_ps = psb.tile([S, G], f32, tag="big", name="ma_ps")
                    nc.tensor.matmul(ma_ps[:], A8t[:], mcur, start=True, stop=False)
                    nc.tensor.matmul(ma_ps[:], Wst[:], yb, start=False, stop=True)
                    mS = spool.tile([S, G], f32, tag="mS", name="mS")
                    nc.vector.tensor_copy(mS[:], ma_ps[:])
                    mcur = mS[:]

    nc.compile()
    return nc


_program_cache = {}


def _get_program(Tcut, Ttot):
    key = (Tcut, Ttot)
    if key not in _program_cache:
        _program_cache[key] = build_program(Tcut, Ttot)
    return _program_cache[key]


def _host_precompute(obs, F, Q, H, R, init_mean, init_cov, Tcut=TCUT, Ttot=T):
    f = np.float32
    nblk = (Ttot - Tcut) // B
    Fsc = np.sqrt(0.5).astype(f) * F.astype(f)
    P0 = init_cov[0].astype(f)
    Qsym = (0.5 * (Q + Q.T)).astype(f)
    Rsym = (0.5 * (R + R.T)).astype(f)
    S0 = (H.astype(np.float64) @ P0.astype(np.float64) @ H.T.astype(np.float64)
          + Rsym.astype(np.float64))
    X0 = np.linalg.inv(S0)
    X0m = (-X0).astype(f)
    I16 = np.eye(M, dtype=f)
    obs_pre = ((-np.sqrt(2.0)).astype(f) * obs.astype(f)).transpose(1, 2, 0)  # [T,M,G]
    ins = {
        "P0": P0,
        "mT0": np.ascontiguousarray(init_mean.astype(f).T),
        "FH": np.ascontiguousarray(np.concatenate([Fsc.T, H.astype(f).T], axis=1)),
        "Ft": np.ascontiguousarray(F.astype(f).T),
        "Ff": np.ascontiguousarray(F.astype(f)),
        "Qsym": Qsym,
        "Hf": np.ascontiguousarray(H.astype(f)),
        "Rsym": Rsym,
        "I22": np.ascontiguousarray(np.concatenate([2 * I16, 2 * I16], axis=1)),
        "X022": np.ascontiguousarray(np.concatenate([X0m, X0m.T], axis=1)),
        "obs_tr": np.ascontiguousarray(obs_pre[:Tcut]),
    }
    if nblk:
        ins["obs_st"] = np.ascontiguousarray(
            obs_pre[Tcut:Ttot].reshape(nblk, B, M, G).transpose(1, 2, 0, 3)
            .reshape(B * M, nblk * G)
        )
    return ins


def _run_device(inputs, trace=False, tmpdir=None, Tcut=TCUT, Ttot=T):
    obs, F, Q, H, R = (inputs["obs"], inputs["F"], inputs["Q"], inputs["H"],
                       inputs["R"])
    init_mean, init_cov = inputs["init_mean"], inputs["init_cov"]
    nblk = (Ttot - Tcut) // B
    nc = _get_program(Tcut, Ttot)
    ins = _host_precompute(obs, F, Q, H, R, init_mean, init_cov, Tcut, Ttot)
    in_maps = [dict(ins) for _ in range(NCORES)]
    res = run_bass_kernel_spmd(
        nc, in_maps, list(range(NCORES)), trace=trace, tmpdir=tmpdir
    )
    out = res.results[0]
    f = np.float32
    means = np.empty((G, Ttot, M), f)
    # transient: meas = resid'/sqrt2 + y
    resid = out["out_resid"]                        # [M, Tcut, G]
    means[:, :Tcut, :] = (resid.transpose(2, 1, 0) / np.sqrt(2.0).astype(f)
                          + obs[:, :Tcut, :].astype(f))
    if nblk:
        mst = out["out_mst"]                        # [nblk, B*M, G]
        means[:, Tcut:, :] = (mst.reshape(nblk, B, M, G).transpose(3, 0, 1, 2)
                              .reshape(G, Ttot - Tcut, M))
    covs_mtm = out["out_covs"]                      # [M, Tcut, M]
    ss = covs_mtm.transpose(1, 0, 2)                # [Tcut, M, M]
    covs_t = np.empty((Ttot, M, M), f)
    covs_t[:Tcut] = ss
    covs_t[Tcut:] = ss[Tcut - 1]
    covs = np.broadcast_to(covs_t[None], (G, Ttot, M, M)).copy()
    return means, covs, res


def _reference_fallback(obs, F, Q, H, R, init_mean, init_cov):
    """Exact per-group filter in float64 — used only if assumptions fail."""
    d = np.float64
    obs, F, Q, H, R = obs.astype(d), F.astype(d), Q.astype(d), H.astype(d), R.astype(d)
    m = init_mean.astype(d)
    P = 0.5 * (init_cov.astype(d) + init_cov.astype(d).transpose(0, 2, 1))
    g, tt, mm_ = obs.shape
    s = F.shape[0]
    means = np.zeros((g, tt, mm_), np.float32)
    covs = np.zeros((g, tt, mm_, mm_), np.float32)
    I = np.eye(s)
    for t in range(tt):
        meas_mean = m @ H.T
        PHt = P @ H.T
        meas_cov = np.einsum("ms,gsn->gmn", H, PHt) + R
        means[:, t] = meas_mean
        covs[:, t] = meas_cov
        Kt = np.linalg.solve(meas_cov, PHt.transpose(0, 2, 1))
        K = Kt.transpose(0, 2, 1)
        resid = obs[:, t] - meas_mean
        m_upd = m + np.einsum("gsm,gm->gs", K, resid)
        P_upd = (I[None] - K @ H) @ P
        m = m_upd @ F.T
        X = np.einsum("ij,gjk,lk->gil", F, P_upd, F) + Q
        P = 0.5 * (X + X.transpose(0, 2, 1))
    return means, covs


def kernel(obs, F, Q, H, R, init_mean, init_cov):
    obs = np.asarray(obs)
    F = np.asarray(F); Q = np.asarray(Q); H = np.asarray(H); R = np.asarray(R)
    init_mean = np.asarray(init_mean); init_cov = np.asarray(init_cov)

    shapes_ok = (
        obs.shape == (G, T, M) and F.shape == (S, S) and H.shape == (M, S)
        and init_cov.shape == (G, S, S)
    )
    uniform_cov = shapes_ok and bool(np.all(init_cov == init_cov[0]))
    if not uniform_cov:
        means, covs = _reference_fallback(obs, F, Q, H, R, init_mean, init_cov)
        return means, covs

    try:
        means, covs, _ = _run_device(
            dict(obs=obs, F=F, Q=Q, H=H, R=R, init_mean=init_mean, init_cov=init_cov)
        )
        if not (np.isfinite(means).all() and np.isfinite(covs).all()):
            raise FloatingPointError("non-finite device output")
    except Exception:
        means, covs = _reference_fallback(obs, F, Q, H, R, init_mean, init_cov)
    return means, covs
